# revision 1
# baseline (speedup 1.0000x reference)
"""Trainium2 Bass kernel for nn_MultiHeadClassifier.

  logits[b, c] = sum_{(g,l): label_ids[g,l]==c} group_probs[b,g] *
                 (features[b] @ W[g,l] + b[g,l])

Data-parallel over batch (8 cores, 4096 rows each). Per core:
  * Host prep: transpose features/group_probs; sort the G*L=1024 head
    outputs by target class, pad so no class straddles a 128-row chunk
    -> NCH chunks with disjoint class bands covering [0, C).
  * GEMM1 (PE, bf16): glT[gl, b] = Wsorted^T.T @ X^T per chunk/b-tile.
  * M-matmul (PE, bf16): MT[gl, b] = E_j.T @ pT (group-prob gather as a
    0/1 matmul).
  * ACT: per-partition bias add + PSUM drain; DVE: weighted = gb * MT.
  * Scatter (PE, bf16): logits[b, lo_j:hi_j] = weightedT_j.T @ S_j with
    S_j a 0/1 band matrix; bands disjoint -> independent start=True
    matmuls, accumulation happens inside the band via duplicate class
    columns of S.
"""
import os
import sys
import numpy as np
import ml_dtypes

for _p in ("/opt/trn_rl_repo",):
    if _p not in sys.path:
        sys.path.append(_p)

import concourse.bass as bass  # noqa: E402
import concourse.tile as tile  # noqa: E402
from concourse import bacc, mybir, bass_utils  # noqa: E402
from contextlib import ExitStack  # noqa: E402

F32 = mybir.dt.float32
BF16 = mybir.dt.bfloat16

B, F, G, L, C = 32768, 512, 16, 64, 1000
NCORE = 8
BC = B // NCORE          # 4096 batch rows per core
NT = BC // 512           # 8 b-tiles of 512
KF = F // 128            # 4 feature chunks

LAST_EXEC_NS = None


def _host_prep(W, b, label_ids):
    lab = np.asarray(label_ids).reshape(-1).astype(np.int64)
    GL = lab.shape[0]
    order = np.argsort(lab, kind="stable")
    rows, cur = [], 0
    classes, starts = np.unique(lab[order], return_index=True)
    starts = list(starts) + [GL]
    for ci in range(len(classes)):
        seg = order[starts[ci]:starts[ci + 1]]
        if cur + len(seg) > 128:
            rows += [-1] * (128 - cur)
            cur = 0
        rows += list(seg)
        cur = (cur + len(seg)) % 128
    if len(rows) % 128:
        rows += [-1] * (128 - len(rows) % 128)
    rows = np.array(rows, dtype=np.int64)
    K_pad = len(rows)
    NCH = K_pad // 128

    his = []
    for j in range(NCH):
        rj = rows[j * 128:(j + 1) * 128]
        valid = rj[rj >= 0]
        his.append(int(lab[valid].max()) + 1 if len(valid) else (his[-1] if his else 0))
    his[-1] = C
    for j in range(1, NCH):
        his[j] = max(his[j], his[j - 1])
    los = [0] + his[:-1]
    bands = list(zip(los, his))

    S_cat = np.zeros((128, C), dtype=ml_dtypes.bfloat16)
    for j, (lo, hi) in enumerate(bands):
        rj = rows[j * 128:(j + 1) * 128]
        for r in range(128):
            gl = rj[r]
            if gl >= 0:
                S_cat[r, lab[gl]] = 1.0

    Wflat = np.asarray(W).reshape(GL, F)
    bflat = np.asarray(b).reshape(GL)
    WT = np.zeros((F, K_pad), dtype=np.float32)
    biasT = np.zeros((128, NCH), dtype=np.float32)
    E = np.zeros((16, K_pad), dtype=ml_dtypes.bfloat16)
    for p, gl in enumerate(rows):
        if gl >= 0:
            WT[:, p] = Wflat[gl]
            biasT[p % 128, p // 128] = bflat[gl]
            E[gl // L, p] = 1.0
    return dict(K_pad=K_pad, NCH=NCH, bands=bands, S_cat=S_cat,
                WT=WT.astype(ml_dtypes.bfloat16), biasT=biasT, E=E)


def _band_segments(lo, hi):
    """Split [lo, hi) at 512-column (PSUM bank) boundaries."""
    segs = []
    while lo < hi:
        nxt = min(hi, (lo // 512 + 1) * 512)
        segs.append((lo, nxt))
        lo = nxt
    return segs


def _build_program(NCH, bands):
    nc = bacc.Bacc("TRN2", target_bir_lowering=False, debug=False,
                   num_devices=NCORE)
    xt_d = nc.dram_tensor("xt", [F, BC], BF16, kind="ExternalInput").ap()
    pt_d = nc.dram_tensor("pt", [16, BC], BF16, kind="ExternalInput").ap()
    wt_d = nc.dram_tensor("wt", [F, NCH * 128], BF16, kind="ExternalInput").ap()
    e_d = nc.dram_tensor("e", [16, NCH * 128], BF16, kind="ExternalInput").ap()
    bt_d = nc.dram_tensor("bt", [128, NCH], F32, kind="ExternalInput").ap()
    s_d = nc.dram_tensor("s", [128, C], BF16, kind="ExternalInput").ap()
    out_d = nc.dram_tensor("logits", [BC, C], F32, kind="ExternalOutput").ap()

    with tile.TileContext(nc) as tc, ExitStack() as ctx:
        const = ctx.enter_context(tc.tile_pool(name="const", bufs=1))
        psG = ctx.enter_context(tc.tile_pool(name="psG", bufs=2, space="PSUM"))
        psM = ctx.enter_context(tc.tile_pool(name="psM", bufs=2, space="PSUM"))
        psL = ctx.enter_context(tc.tile_pool(name="psL", bufs=2, space="PSUM"))
        sbG = ctx.enter_context(tc.tile_pool(name="sbG", bufs=6))
        sbW = ctx.enter_context(tc.tile_pool(name="sbW", bufs=24))
        sbO = ctx.enter_context(tc.tile_pool(name="sbO", bufs=6))

        # X^T column-slices: separate tiles for precise DMA deps.
        xts = [[None] * NT for _ in range(KF)]

        def load_x(k, t):
            t_ = const.tile([128, 512], BF16, name=f"x{k}_{t}", tag=f"x{k}_{t}")
            nc.sync.dma_start(t_[:],
                              xt_d[k * 128:(k + 1) * 128, bass.ts(t, 512)])
            xts[k][t] = t_

        # interleave the tiles needed by the first GEMM (x slices of t=0 and
        # W chunks) so the PE can start as early as possible
        wts = []
        for k in range(KF):
            load_x(k, 0)
            t_ = const.tile([128, NCH * 128], BF16, name=f"wts{k}", tag=f"wts{k}")
            nc.gpsimd.dma_start(t_[:], wt_d[k * 128:(k + 1) * 128, :])
            wts.append(t_)
        pts = const.tile([16, BC], BF16, name="pts", tag="pts")
        nc.gpsimd.dma_start(pts[:], pt_d[:])
        es = const.tile([16, NCH * 128], BF16, name="es", tag="es")
        nc.gpsimd.dma_start(es[:], e_d[:])
        bts = const.tile([128, NCH], F32, name="bts", tag="bts")
        nc.gpsimd.dma_start(bts[:], bt_d[:])
        ss = const.tile([128, C], BF16, name="ss", tag="ss")
        nc.gpsimd.dma_start(ss[:], s_d[:])
        for t in range(1, NT):
            for k in range(KF):
                load_x(k, t)

        all_wtjs = {}

        def gemm_phase(t):
            bsl = bass.ts(t, 512)
            wtjs = []
            for j in range(NCH):
                jsl = bass.ts(j, 128)
                pg = psG.tile([128, 512], F32, name="pg", tag="pg")
                for k in range(KF):
                    nc.tensor.matmul(pg[:], wts[k][:, jsl], xts[k][t][:],
                                     start=(k == 0), stop=(k == KF - 1))
                pm = psM.tile([128, 512], F32, name="pm", tag="pm")
                nc.tensor.matmul(pm[:], es[:, jsl], pts[:, bsl],
                                 start=True, stop=True)
                gb = sbG.tile([128, 512], BF16, name="gb", tag="gb")
                nc.scalar.activation(gb[:], pg[:],
                                     mybir.ActivationFunctionType.Identity,
                                     bias=bts[:, j:j + 1], scale=1.0)
                wtj = sbW.tile([128, 512], BF16, name="wtj", tag="wtj")
                nc.vector.tensor_mul(wtj[:], gb[:], pm[:])
                wtjs.append(wtj)
            all_wtjs[t] = wtjs

        def scatter_phase(t):
            wtjs = all_wtjs.pop(t)
            for bs_i in range(4):
                pl = psL.tile([128, 1024], F32, name="pl", tag="pl")
                for j, (lo, hi) in enumerate(bands):
                    for (n0, n1) in _band_segments(lo, hi):
                        nc.tensor.matmul(pl[:, n0:n1],
                                         wtjs[j][:, bass.ts(bs_i, 128)],
                                         ss[:, n0:n1], start=True, stop=True)
                ob = sbO.tile([128, C], F32, name="ob", tag="ob")
                # split the PSUM drain per bank across both engines
                nc.scalar.activation(ob[:, :512], pl[:, :512],
                                     mybir.ActivationFunctionType.Identity,
                                     bias=0.0, scale=1.0)
                nc.vector.tensor_copy(ob[:, 512:C], pl[:, 512:C])
                # scalar-queue HWDGE: keep output stream off the input queue
                nc.scalar.dma_start(out_d[t * 512 + bs_i * 128:
                                          t * 512 + (bs_i + 1) * 128, :], ob[:])

        # software-pipelined emission: scatter(t-1) after gemm(t)
        for t in range(NT + 1):
            if t < NT:
                gemm_phase(t)
            if t > 0:
                scatter_phase(t - 1)
    nc.finalize()
    return nc


def kernel(features, group_probs, W, b, label_ids):
    global LAST_EXEC_NS
    features = np.asarray(features, dtype=np.float32)
    group_probs = np.asarray(group_probs, dtype=np.float32)
    prep = _host_prep(W, b, label_ids)
    nc = _build_program(prep["NCH"], prep["bands"])

    XT = np.ascontiguousarray(features.T.astype(ml_dtypes.bfloat16))
    PT = np.ascontiguousarray(group_probs.T.astype(ml_dtypes.bfloat16))
    in_maps = []
    for c in range(NCORE):
        in_maps.append({
            "xt": np.ascontiguousarray(XT[:, c * BC:(c + 1) * BC]),
            "pt": np.ascontiguousarray(PT[:, c * BC:(c + 1) * BC]),
            "wt": prep["WT"],
            "e": prep["E"],
            "bt": prep["biasT"],
            "s": prep["S_cat"],
        })

    trace = bool(os.environ.get("BASS_TRACE"))
    if trace:
        bass_utils.upload_artifacts = lambda d: "local://skipped"
    try:
        res = bass_utils.run_bass_kernel_spmd(nc, in_maps,
                                              core_ids=list(range(NCORE)))
    except Exception:
        # transient NRT device errors have been observed; one retry
        res = bass_utils.run_bass_kernel_spmd(nc, in_maps,
                                              core_ids=list(range(NCORE)))
    if trace:
        LAST_EXEC_NS = res.exec_time_ns
        if res.exec_time_ns is not None:
            print(f"HW exec time: {res.exec_time_ns} ns")

    out = np.concatenate([res.results[c]["logits"] for c in range(NCORE)],
                         axis=0)
    return np.ascontiguousarray(out.astype(np.float32))

